# revision 9
# baseline (speedup 1.0000x reference)
import sys

if "/opt/trn_rl_repo" not in sys.path:
    sys.path.insert(0, "/opt/trn_rl_repo")

import numpy as np
import ml_dtypes

from concourse import bass, tile, bacc
from concourse.bass import mybir

F32 = mybir.dt.float32
F16 = mybir.dt.float16
BF16 = mybir.dt.bfloat16
I16 = mybir.dt.int16

N_CORES = 8
N_TOTAL = 32768
N_CORE = N_TOTAL // N_CORES  # 4096 rows per core
D = 1024
C = 64
K = 16
DEPTH = 4
M = 1024
STAGES = [256, 256, 512, 512, 512, 1024, 1024]  # rows per stage (sum = N_CORE)
N_WARM_MM = 16                  # dummy matmuls bridging the prologue
ALU = mybir.AluOpType
AFT = mybir.ActivationFunctionType

assert sum(STAGES) == N_CORE
_bases = np.cumsum([0] + STAGES[:-1]).tolist()
_choff = np.cumsum([0] + [DEPTH * (w // 2) for w in STAGES[:-1]]).tolist()
CH_COLS = sum(DEPTH * (w // 2) for w in STAGES)  # 8192


def build_program(repeat=1):
    nc = bacc.Bacc()
    # chd[64*h+c, off_s + d*HW + n'] = x[base_s + h*HW + n', dims[4c+d]]
    chd_d = nc.declare_dram_parameter("chd", [128, CH_COLS], F32, isOutput=False)
    thr_d = nc.declare_dram_parameter("thrcols", [128, 15], F32, isOutput=False)
    lut_d = nc.declare_dram_parameter("lutT", [C * K, M], BF16, isOutput=False)
    ktab_d = nc.declare_dram_parameter("ktab", [128, 8], F32, isOutput=False)
    out_d = nc.declare_dram_parameter("out", [N_CORE, M], F16, isOutput=True)

    with tile.TileContext(nc) as tc:
        from contextlib import ExitStack
        es = ExitStack()
        sb = es.enter_context(tc.tile_pool(name="sb", bufs=1))
        pspool = es.enter_context(
            tc.tile_pool(name="ps", bufs=4, space=bass.MemorySpace.PSUM)
        )

        WMAX = max(STAGES)

        # ---- persistent tiles ----
        luts = [sb.tile([128, M], BF16, name=f"lut{j}", tag=f"lut{j}")
                for j in range(8)]
        thr = sb.tile([128, 15], F32, name="thr_sb", tag="thr")
        ktab = sb.tile([128, 8], F32, name="ktab_sb", tag="ktab")
        wdum = sb.tile([128, 512], BF16, name="wdum", tag="wdum")
        tmps = [sb.tile([128, WMAX // 2], F32, name=f"tmp{ti}", tag=f"tmp{ti}")
                for ti in range(7)]
        b0, b1, b2, sa, sb_, sc, sd = tmps
        bi = sb.tile([128, WMAX], I16, name="bi_sb", tag="bi")
        b0i = bi[:, :WMAX // 2]
        b1i = bi[:, WMAX // 2:]

        # warmup lhs has no DMA dependency: memset locally (gpsimd is idle)
        nc.gpsimd.memset(wdum[:], 0.0)

        # small constants first on scalar queue (kept uncongested for b2k)
        nc.scalar.dma_start(thr[:], thr_d[:])
        nc.scalar.dma_start(ktab[:], ktab_d[:])

        # stage-0 chosen-x block first on sync queue (critical path)
        ch_tiles = [
            sb.tile([128, DEPTH, W // 2], F32, name=f"ch{s}", tag=f"ch{s}")
            for s, W in enumerate(STAGES)
        ]
        def load_ch(s, eng):
            HW_ = STAGES[s] // 2
            eng.dma_start(
                ch_tiles[s][:], chd_d[:, _choff[s]:_choff[s] + DEPTH * HW_]
            )

        # upfront: stage 0/1 inputs + lut split across both queues; the rest
        # is prefetched from inside the stage loop so the issuing engines
        # never build a long blocked run of DMA_DIRECT2D instructions.
        load_ch(0, nc.sync)
        for j in range(4):
            nc.sync.dma_start(luts[j][:], lut_d[j * 128:(j + 1) * 128, :])
        for j in range(4, 8):
            nc.scalar.dma_start(luts[j][:], lut_d[j * 128:(j + 1) * 128, :])
        load_ch(1, nc.sync)
        load_ch(2, nc.sync)

        # dummy matmuls keep the PE clock un-throttled through the prologue
        wps = pspool.tile([128, 512], F32, name="wps", tag="ps0")
        for _ in range(N_WARM_MM):
            nc.tensor.matmul(wps[:], wdum[:, 0:128], wdum[:],
                             start=True, stop=True)

        def tcol(i):
            return thr[:, i:i + 1]

        from concourse.tile import add_dep_helper
        last_iseq = None
        stage_list = [sw for _ in range(repeat) for sw in zip(range(len(STAGES)),
                                                              STAGES, _bases)]
        for s, W, base in stage_list:
            HW_ = W // 2  # half-stage width
            ch = ch_tiles[s]
            xd = [ch[:, d, :] for d in range(DEPTH)]
            if s + 3 < len(STAGES):
                load_ch(s + 3, nc.sync)

            def T(t):
                return t[:, :HW_]

            # ---- tree descent on [128=(h,c), HW_] ----
            i0 = nc.vector.tensor_scalar(T(b0), xd[0], tcol(0), None, ALU.is_gt)
            if last_iseq is not None:
                add_dep_helper(i0.ins, last_iseq.ins, sync=False,
                               reason="DVE order: ET compares before next descent")
            nc.vector.tensor_scalar(T(sa), T(b0), tcol(2), tcol(1), ALU.mult, ALU.add)
            nc.vector.tensor_copy(T(b0i), T(b0))
            nc.vector.tensor_tensor(T(b1), xd[1], T(sa), ALU.is_gt)

            nc.vector.tensor_scalar(T(sa), T(b1), tcol(4), tcol(3), ALU.mult, ALU.add)
            nc.vector.tensor_scalar(T(sb_), T(b1), tcol(6), tcol(5), ALU.mult, ALU.add)
            nc.vector.tensor_copy(T(b1i), T(b1))
            nc.vector.copy_predicated(T(sa), T(b0i), T(sb_))
            nc.vector.tensor_tensor(T(b2), xd[2], T(sa), ALU.is_gt)

            nc.vector.tensor_scalar(T(sa), T(b2), tcol(8), tcol(7), ALU.mult, ALU.add)
            nc.vector.tensor_scalar(T(sb_), T(b2), tcol(10), tcol(9), ALU.mult, ALU.add)
            nc.vector.tensor_scalar(T(sc), T(b2), tcol(12), tcol(11), ALU.mult, ALU.add)
            nc.vector.tensor_scalar(T(sd), T(b2), tcol(14), tcol(13), ALU.mult, ALU.add)
            nc.vector.copy_predicated(T(sa), T(b1i), T(sb_))
            nc.vector.copy_predicated(T(sc), T(b1i), T(sd))
            nc.vector.copy_predicated(T(sa), T(b0i), T(sc))
            nc.vector.tensor_tensor(T(sb_), xd[3], T(sa), ALU.is_gt)  # b3 -> sb_

            bk = sb.tile([128, HW_], BF16, name="bk", tag="bk", bufs=2)
            nc.vector.scalar_tensor_tensor(T(sc), T(b0), 2.0, T(b1), ALU.mult, ALU.add)
            nc.vector.scalar_tensor_tensor(T(sd), T(sc), 2.0, T(b2), ALU.mult, ALU.add)
            nc.vector.scalar_tensor_tensor(bk[:], T(sd), 2.0, T(sb_), ALU.mult, ALU.add)

            # ---- duplicate bucket to both e-halves: b2k[64e+c, h*HW_+n'] ----
            b2k = sb.tile([128, W], BF16, name="b2k", tag="b2k", bufs=2)
            for e in range(2):
                for h in range(2):
                    _eng = nc.scalar if (e + h) % 2 else nc.sync
                    _eng.dma_start(
                        b2k[64 * e:64 * e + 64, h * HW_:(h + 1) * HW_],
                        bk[64 * h:64 * h + 64, :],
                    )

            # ---- ET: et[p=(e,c), tau, nn] = (bucket == 2*tau + e) ----
            # stages >=2 run ET on the otherwise-idle gpsimd engine so the
            # DVE descent stream stays ahead of the PE
            et_eng = nc.vector if s < 2 else nc.gpsimd
            et = sb.tile([128, 8, W], BF16, name="et", tag="et", bufs=2)
            for tau in range(8):
                iseq = et_eng.tensor_scalar(
                    et[:, tau, :], b2k[:], ktab[:, tau:tau + 1], None, ALU.is_equal
                )
                if s < 2:
                    last_iseq = iseq

            # ---- matmul + output ----
            for i in range(W // 128):
                ps = [
                    pspool.tile([128, 512], F32, name=f"ps{mc}", tag=f"ps{mc}")
                    for mc in range(2)
                ]
                for tau in range(8):
                    lhsT = et[:, tau, i * 128:(i + 1) * 128]
                    for mc in range(2):
                        nc.tensor.matmul(
                            ps[mc][:], lhsT, luts[tau][:, mc * 512:(mc + 1) * 512],
                            start=(tau == 0), stop=(tau == 7),
                        )
                osb = sb.tile([128, M], F16, name="osb", tag="osb", bufs=4)
                nc.scalar.activation(osb[:, 0:512], ps[0][:], AFT.Copy)
                nc.scalar.activation(osb[:, 512:1024], ps[1][:], AFT.Copy)
                r0 = base + i * 128
                nc.sync.dma_start(out_d[r0:r0 + 128, :], osb[:])
        es.close()
    nc.finalize()
    return nc


def _prep_inputs(inputMatrix, dims, thresholds, lut):
    x = np.asarray(inputMatrix, dtype=np.float32)
    dims_a = np.asarray(dims).ravel().astype(np.int64).reshape(C, DEPTH)
    thr = np.asarray(thresholds, dtype=np.float32).reshape(C, K - 1)
    lut = np.asarray(lut, dtype=np.float32)

    # thrcols [128, 15]: t0,t1,d21,t3,d43,t5,d65,t7,d87,t9,d109,t11,d1211,t13,d1413
    tcols = np.empty((C, 15), dtype=np.float32)
    tcols[:, 0] = thr[:, 0]
    pairs = [(1, 2), (3, 4), (5, 6), (7, 8), (9, 10), (11, 12), (13, 14)]
    for idx, (lo, hi) in enumerate(pairs):
        tcols[:, 1 + 2 * idx] = thr[:, lo]
        tcols[:, 2 + 2 * idx] = thr[:, hi] - thr[:, lo]
    thrcols = np.concatenate([tcols, tcols], axis=0)  # [128, 15]

    # lutT row tau*128 + 64e + c -> lut[m, c, 2*tau+e]
    lt = lut.reshape(M, C, 8, 2).transpose(2, 3, 1, 0).reshape(C * K, M)
    lutT = lt.astype(ml_dtypes.bfloat16)

    # ktab[p, tau] = 2*tau + p//64
    ktab = (2 * np.arange(8)[None, :] + (np.arange(128) // 64)[:, None]
            ).astype(np.float32)

    # chd per core: [64h+c, off_s + d*HW + n'] = x_shard[base+h*HW+n', dims[c,d]]
    chd = np.empty((N_CORES, 128, CH_COLS), dtype=np.float32)
    for i in range(N_CORES):
        xs = x[i * N_CORE:(i + 1) * N_CORE]
        for s, (W, base) in enumerate(zip(STAGES, _bases)):
            HW_ = W // 2
            blk = xs[base:base + W][:, dims_a]          # [W, C, DEPTH]
            blk = blk.reshape(2, HW_, C, DEPTH).transpose(0, 2, 3, 1)
            chd[i, :, _choff[s]:_choff[s] + DEPTH * HW_] = blk.reshape(
                128, DEPTH * HW_)

    return chd, thrcols, lutT, ktab


def _make_in_maps(chd, thrcols, lutT, ktab):
    return [
        {
            "chd": np.ascontiguousarray(chd[i]),
            "thrcols": thrcols,
            "lutT": lutT,
            "ktab": ktab,
        }
        for i in range(N_CORES)
    ]


def kernel(inputMatrix, dims, thresholds, lut, selection_matrix=None,
           tree_des_mat=None):
    from concourse.bass_utils import run_bass_kernel_spmd

    prep = _prep_inputs(inputMatrix, dims, thresholds, lut)
    nc = build_program()
    in_maps = _make_in_maps(*prep)
    res = run_bass_kernel_spmd(nc, in_maps, list(range(N_CORES)))
    out = np.concatenate(
        [np.asarray(res.results[i]["out"]) for i in range(N_CORES)], axis=0
    )
    return out.astype(np.float32)


# revision 10
# speedup vs baseline: 3.4497x; 3.4497x over previous
import sys

if "/opt/trn_rl_repo" not in sys.path:
    sys.path.insert(0, "/opt/trn_rl_repo")

import numpy as np
import ml_dtypes

from concourse import bass, tile, bacc
from concourse.bass import mybir

F32 = mybir.dt.float32
F16 = mybir.dt.float16
BF16 = mybir.dt.bfloat16
I16 = mybir.dt.int16

N_CORES = 8
N_TOTAL = 32768
N_CORE = N_TOTAL // N_CORES  # 4096 rows per core
D = 1024
C = 64
K = 16
DEPTH = 4
M = 1024
STAGES = [256, 256, 512, 512, 512, 1024, 1024]  # rows per stage (sum = N_CORE)
N_WARM_MM = 16                  # dummy matmuls bridging the prologue
ALU = mybir.AluOpType
AFT = mybir.ActivationFunctionType

assert sum(STAGES) == N_CORE
_bases = np.cumsum([0] + STAGES[:-1]).tolist()
_choff = np.cumsum([0] + [DEPTH * (w // 2) for w in STAGES[:-1]]).tolist()
CH_COLS = sum(DEPTH * (w // 2) for w in STAGES)  # 8192


def build_program(repeat=1):
    nc = bacc.Bacc()
    # chd[64*h+c, off_s + d*HW + n'] = x[base_s + h*HW + n', dims[4c+d]]
    chd_d = nc.declare_dram_parameter("chd", [128, CH_COLS], F32, isOutput=False)
    thr_d = nc.declare_dram_parameter("thrcols", [128, 15], F32, isOutput=False)
    lut_d = nc.declare_dram_parameter("lutT", [C * K, M], BF16, isOutput=False)
    ktab_d = nc.declare_dram_parameter("ktab", [128, 8], F32, isOutput=False)
    out_d = nc.declare_dram_parameter("out", [N_CORE, M], F16, isOutput=True)

    with tile.TileContext(nc) as tc:
        from contextlib import ExitStack
        es = ExitStack()
        sb = es.enter_context(tc.tile_pool(name="sb", bufs=1))
        pspool = es.enter_context(
            tc.tile_pool(name="ps", bufs=4, space=bass.MemorySpace.PSUM)
        )

        WMAX = max(STAGES)

        # ---- persistent tiles ----
        luts = [sb.tile([128, M], BF16, name=f"lut{j}", tag=f"lut{j}")
                for j in range(8)]
        thr = sb.tile([128, 15], F32, name="thr_sb", tag="thr")
        ktab = sb.tile([128, 8], F32, name="ktab_sb", tag="ktab")
        wdum = sb.tile([128, 512], BF16, name="wdum", tag="wdum")
        tmps = [sb.tile([128, WMAX // 2], F32, name=f"tmp{ti}", tag=f"tmp{ti}")
                for ti in range(7)]
        b0, b1, b2, sa, sb_, sc, sd = tmps
        bi = sb.tile([128, WMAX], I16, name="bi_sb", tag="bi")
        b0i = bi[:, :WMAX // 2]
        b1i = bi[:, WMAX // 2:]

        # warmup lhs has no DMA dependency: memset locally (gpsimd is idle)
        nc.gpsimd.memset(wdum[:], 0.0)

        # small constants first on scalar queue (kept uncongested for b2k)
        nc.scalar.dma_start(thr[:], thr_d[:])
        nc.scalar.dma_start(ktab[:], ktab_d[:])

        # stage-0 chosen-x block first on sync queue (critical path)
        ch_tiles = [
            sb.tile([128, DEPTH, W // 2], F32, name=f"ch{s}", tag=f"ch{s}")
            for s, W in enumerate(STAGES)
        ]
        def load_ch(s, eng):
            HW_ = STAGES[s] // 2
            eng.dma_start(
                ch_tiles[s][:], chd_d[:, _choff[s]:_choff[s] + DEPTH * HW_]
            )

        # upfront: stage 0/1 inputs + lut split across both queues; the rest
        # is prefetched from inside the stage loop so the issuing engines
        # never build a long blocked run of DMA_DIRECT2D instructions.
        load_ch(0, nc.sync)
        for j in range(4):
            nc.sync.dma_start(luts[j][:], lut_d[j * 128:(j + 1) * 128, :])
        for j in range(4, 8):
            nc.scalar.dma_start(luts[j][:], lut_d[j * 128:(j + 1) * 128, :])
        load_ch(1, nc.sync)
        load_ch(2, nc.sync)

        # dummy matmuls keep the PE clock un-throttled through the prologue
        wps = pspool.tile([128, 512], F32, name="wps", tag="ps0")
        for _ in range(N_WARM_MM):
            nc.tensor.matmul(wps[:], wdum[:, 0:128], wdum[:],
                             start=True, stop=True)

        def tcol(i):
            return thr[:, i:i + 1]

        from concourse.tile import add_dep_helper
        last_iseq = None
        stage_list = [sw for _ in range(repeat) for sw in zip(range(len(STAGES)),
                                                              STAGES, _bases)]
        for s, W, base in stage_list:
            HW_ = W // 2  # half-stage width
            ch = ch_tiles[s]
            xd = [ch[:, d, :] for d in range(DEPTH)]
            if s + 3 < len(STAGES):
                load_ch(s + 3, nc.sync)

            def T(t):
                return t[:, :HW_]

            # ---- tree descent on [128=(h,c), HW_] ----
            i0 = nc.vector.tensor_scalar(T(b0), xd[0], tcol(0), None, ALU.is_gt)
            if last_iseq is not None:
                add_dep_helper(i0.ins, last_iseq.ins, sync=False,
                               reason="DVE order: ET compares before next descent")
            nc.vector.tensor_scalar(T(sa), T(b0), tcol(2), tcol(1), ALU.mult, ALU.add)
            nc.vector.tensor_copy(T(b0i), T(b0))
            nc.vector.tensor_tensor(T(b1), xd[1], T(sa), ALU.is_gt)

            nc.vector.tensor_scalar(T(sa), T(b1), tcol(4), tcol(3), ALU.mult, ALU.add)
            nc.vector.tensor_scalar(T(sb_), T(b1), tcol(6), tcol(5), ALU.mult, ALU.add)
            nc.vector.tensor_copy(T(b1i), T(b1))
            nc.vector.copy_predicated(T(sa), T(b0i), T(sb_))
            nc.vector.tensor_tensor(T(b2), xd[2], T(sa), ALU.is_gt)

            nc.vector.tensor_scalar(T(sa), T(b2), tcol(8), tcol(7), ALU.mult, ALU.add)
            nc.vector.tensor_scalar(T(sb_), T(b2), tcol(10), tcol(9), ALU.mult, ALU.add)
            nc.vector.tensor_scalar(T(sc), T(b2), tcol(12), tcol(11), ALU.mult, ALU.add)
            nc.vector.tensor_scalar(T(sd), T(b2), tcol(14), tcol(13), ALU.mult, ALU.add)
            nc.vector.copy_predicated(T(sa), T(b1i), T(sb_))
            nc.vector.copy_predicated(T(sc), T(b1i), T(sd))
            nc.vector.copy_predicated(T(sa), T(b0i), T(sc))
            nc.vector.tensor_tensor(T(sb_), xd[3], T(sa), ALU.is_gt)  # b3 -> sb_

            bk = sb.tile([128, HW_], BF16, name="bk", tag="bk", bufs=2)
            nc.vector.scalar_tensor_tensor(T(sc), T(b0), 2.0, T(b1), ALU.mult, ALU.add)
            nc.vector.scalar_tensor_tensor(T(sd), T(sc), 2.0, T(b2), ALU.mult, ALU.add)
            nc.vector.scalar_tensor_tensor(bk[:], T(sd), 2.0, T(sb_), ALU.mult, ALU.add)

            # ---- duplicate bucket to both e-halves: b2k[64e+c, h*HW_+n'] ----
            b2k = sb.tile([128, W], BF16, name="b2k", tag="b2k", bufs=2)
            for e in range(2):
                for h in range(2):
                    _eng = nc.scalar if (e + h) % 2 else nc.sync
                    _eng.dma_start(
                        b2k[64 * e:64 * e + 64, h * HW_:(h + 1) * HW_],
                        bk[64 * h:64 * h + 64, :],
                    )

            # ---- ET: et[p=(e,c), tau, nn] = (bucket == 2*tau + e) ----
            et = sb.tile([128, 8, W], BF16, name="et", tag="et", bufs=2)
            for tau in range(8):
                last_iseq = nc.vector.tensor_scalar(
                    et[:, tau, :], b2k[:], ktab[:, tau:tau + 1], None, ALU.is_equal
                )

            # ---- matmul + output ----
            for i in range(W // 128):
                ps = [
                    pspool.tile([128, 512], F32, name=f"ps{mc}", tag=f"ps{mc}")
                    for mc in range(2)
                ]
                for tau in range(8):
                    lhsT = et[:, tau, i * 128:(i + 1) * 128]
                    for mc in range(2):
                        nc.tensor.matmul(
                            ps[mc][:], lhsT, luts[tau][:, mc * 512:(mc + 1) * 512],
                            start=(tau == 0), stop=(tau == 7),
                        )
                osb = sb.tile([128, M], F16, name="osb", tag="osb", bufs=4)
                nc.scalar.activation(osb[:, 0:512], ps[0][:], AFT.Copy)
                nc.scalar.activation(osb[:, 512:1024], ps[1][:], AFT.Copy)
                r0 = base + i * 128
                nc.sync.dma_start(out_d[r0:r0 + 128, :], osb[:])
        es.close()
    nc.finalize()
    return nc


def _prep_inputs(inputMatrix, dims, thresholds, lut):
    x = np.asarray(inputMatrix, dtype=np.float32)
    dims_a = np.asarray(dims).ravel().astype(np.int64).reshape(C, DEPTH)
    thr = np.asarray(thresholds, dtype=np.float32).reshape(C, K - 1)
    lut = np.asarray(lut, dtype=np.float32)

    # thrcols [128, 15]: t0,t1,d21,t3,d43,t5,d65,t7,d87,t9,d109,t11,d1211,t13,d1413
    tcols = np.empty((C, 15), dtype=np.float32)
    tcols[:, 0] = thr[:, 0]
    pairs = [(1, 2), (3, 4), (5, 6), (7, 8), (9, 10), (11, 12), (13, 14)]
    for idx, (lo, hi) in enumerate(pairs):
        tcols[:, 1 + 2 * idx] = thr[:, lo]
        tcols[:, 2 + 2 * idx] = thr[:, hi] - thr[:, lo]
    thrcols = np.concatenate([tcols, tcols], axis=0)  # [128, 15]

    # lutT row tau*128 + 64e + c -> lut[m, c, 2*tau+e]
    lt = lut.reshape(M, C, 8, 2).transpose(2, 3, 1, 0).reshape(C * K, M)
    lutT = lt.astype(ml_dtypes.bfloat16)

    # ktab[p, tau] = 2*tau + p//64
    ktab = (2 * np.arange(8)[None, :] + (np.arange(128) // 64)[:, None]
            ).astype(np.float32)

    # chd per core: [64h+c, off_s + d*HW + n'] = x_shard[base+h*HW+n', dims[c,d]]
    chd = np.empty((N_CORES, 128, CH_COLS), dtype=np.float32)
    for i in range(N_CORES):
        xs = x[i * N_CORE:(i + 1) * N_CORE]
        for s, (W, base) in enumerate(zip(STAGES, _bases)):
            HW_ = W // 2
            blk = xs[base:base + W][:, dims_a]          # [W, C, DEPTH]
            blk = blk.reshape(2, HW_, C, DEPTH).transpose(0, 2, 3, 1)
            chd[i, :, _choff[s]:_choff[s] + DEPTH * HW_] = blk.reshape(
                128, DEPTH * HW_)

    return chd, thrcols, lutT, ktab


def _make_in_maps(chd, thrcols, lutT, ktab):
    return [
        {
            "chd": np.ascontiguousarray(chd[i]),
            "thrcols": thrcols,
            "lutT": lutT,
            "ktab": ktab,
        }
        for i in range(N_CORES)
    ]


def kernel(inputMatrix, dims, thresholds, lut, selection_matrix=None,
           tree_des_mat=None):
    from concourse.bass_utils import run_bass_kernel_spmd

    prep = _prep_inputs(inputMatrix, dims, thresholds, lut)
    nc = build_program()
    in_maps = _make_in_maps(*prep)
    res = run_bass_kernel_spmd(nc, in_maps, list(range(N_CORES)))
    out = np.concatenate(
        [np.asarray(res.results[i]["out"]) for i in range(N_CORES)], axis=0
    )
    return out.astype(np.float32)
